# revision 23
# baseline (speedup 1.0000x reference)
# Trainium2 Bass kernel for nn_BottleNeck (sparse local attention bottleneck).
#
# Sharding: data-parallel over batch (B=8 -> 8 cores, one image each).
# BatchNorm batch-statistics are computed as per-core partials and combined
# with three tiny (1-2KB) AllReduce collectives.
#
# On-chip layout: channels on partitions, hw=32*32=1024 on the free dim.
# Channels are PERMUTED so that partitions 0:63 hold the "x-type" attention
# channels (rel depends only on kh) of all 8 groups and 64:127 the "y-type"
# (rel depends only on kw). The permutation is folded into W1/Wq/Wk/Wv/W3 and
# the BN parameters host-side; conv3 un-permutes, so the output is in the
# original channel order.
#
# Attention: for each of the 49 window shifts, l = (k_shift + rel)*q is built
# from shifted AP views of the padded k field (no unfold materialization).
# The (k+rel) add is split across engines for load balance (DVE tensor_scalar
# 4x for kw 2/4/6... see ACT_ADD_KWS / STT_KWS), the *q multiply runs as
# parity-batched bf16 tensor_tensor at DVE 2x (odd shifts read a one-element-
# shifted copy of the field so packed reads stay 4B-aligned), exp runs on the
# scalar engine (1x), and the softmax reductions over the 49 shifts (Z = sum e,
# S = sum e*v) are accumulated on the otherwise-idle TensorEngine as identity
# matmuls into PSUM banks. BN uses exp(-0.5*ln(var+eps)) so the whole kernel
# stays in one ACT table set (no 2.7us table reloads).
#
# Measured (8-core axon trn2, steady-state in-NEFF replication slope):
# ~175-190 us/invocation wall on the slowest core; rel err ~5e-3 vs the fp32
# reference (dominated by bf16 weight quantization).

import os
from contextlib import ExitStack

import numpy as np
import ml_dtypes

import concourse.bass as bass
import concourse.mybir as mybir
import concourse.tile as tile
from concourse import bacc
from concourse.ap import AP
from concourse.bass_utils import run_bass_kernel_spmd

F32 = mybir.dt.float32
BF16 = mybir.dt.bfloat16
AF = mybir.ActivationFunctionType
OP = mybir.AluOpType
AX = mybir.AxisListType

B, C_IN, H, W = 8, 512, 32, 32
PLANES, GROUPS, KS, PAD = 128, 8, 7, 3
D = PLANES // GROUPS
REL = D // 2
HW = H * W
PW = W + 2 * PAD            # 38
PHW = PW * PW               # 1444
EPS = 1e-5
N_CORES = 8
NSAMP = float(B * HW)       # batchnorm sample count over (N,H,W)

# kw plane order inside the per-kh buffers: evens first, then odds, so that
# one AP (kw step 2) covers each parity block contiguously.
KW_ORDER = [0, 2, 4, 6, 1, 3, 5]

# How many of the 49 (k+rel) adds run on ScalarE instead of VectorE (engine
# load balancing; DVE carries the two big multiply passes).
_act_adds = os.environ.get("BASS_ACT_ADDS", "0,1,3")
ACT_ADD_KWS = {int(v) for v in _act_adds.split(",") if v != ""}
_pool_adds = os.environ.get("BASS_POOL_ADDS", "")
POOL_ADD_KWS = {int(v) for v in _pool_adds.split(",") if v != ""}
_CC_MODE = os.environ.get("BASS_CC_MODE", "ag")    # ag=AllGather+local sum, ar=AllReduce
_NO_CC = os.environ.get("BASS_NO_CC") == "1"       # debug: skip collectives
_REPS = int(os.environ.get("BASS_REPS", "1"))      # bench: repeat body in-NEFF
_NO_ATT = os.environ.get("BASS_NO_ATT") == "1"     # debug: skip attention loop


def _sview(flat_ap, off, dims):
    """Hand-built strided view of an SBUF tile ([partition] + dims)."""
    return AP(flat_ap.tensor, off, [list(flat_ap.ap[0])] + [list(d) for d in dims])


def _build_nc():
    nc = bacc.Bacc("TRN2", target_bir_lowering=False, debug=False,
                   num_devices=N_CORES)

    xf_d = nc.dram_tensor("xf", [C_IN, HW], F32, kind="ExternalInput")
    xb_d = nc.dram_tensor("xb", [C_IN, HW], BF16, kind="ExternalInput")
    w1t_d = nc.dram_tensor("w1t", [C_IN, PLANES], BF16, kind="ExternalInput")
    mqkv_d = nc.dram_tensor("mqkv", [3, PLANES, PLANES], BF16, kind="ExternalInput")
    w3t_d = nc.dram_tensor("w3t", [PLANES, 4 * PLANES], BF16, kind="ExternalInput")
    bkv_d = nc.dram_tensor("bkv", [PLANES, 2], F32, kind="ExternalInput")
    relc_d = nc.dram_tensor("relc", [PLANES, KS * KS], F32, kind="ExternalInput")
    gb12_d = nc.dram_tensor("gb12", [PLANES, 4], F32, kind="ExternalInput")
    gb3_d = nc.dram_tensor("gb3", [PLANES, 8], F32, kind="ExternalInput")
    id_d = nc.dram_tensor("id128", [PLANES, PLANES], BF16, kind="ExternalInput")
    out_d = nc.dram_tensor("out", [C_IN, HW], F32, kind="ExternalOutput")

    dbg = os.environ.get("BASS_KDBG") == "1"
    if dbg:
        dbg_d = {n: nc.dram_tensor(f"dbg_{n}", shp, F32, kind="ExternalOutput")
                 for n, shp in [("o1", [128, HW]), ("x1p", [128, PHW]),
                                ("q", [128, HW]), ("kf", [128, PHW]),
                                ("eb0", [128, 7 * HW]), ("z", [128, HW]),
                                ("s", [128, HW]), ("att", [128, HW]),
                                ("o3", [128, 4 * HW])]}

    with tile.TileContext(nc) as tc, ExitStack() as ctx:
        const = ctx.enter_context(tc.tile_pool(name="const", bufs=1))
        sb = ctx.enter_context(tc.tile_pool(name="sb", bufs=1))
        work = ctx.enter_context(tc.tile_pool(name="work", bufs=2))
        psum = ctx.enter_context(tc.tile_pool(name="psum", bufs=1, space="PSUM"))
        dram = ctx.enter_context(tc.tile_pool(name="dram", bufs=1, space="DRAM"))

        # ---------------- constants / weights ----------------
        id_sb = const.tile([128, 128], BF16)
        nc.sync.dma_start(id_sb[:], id_d[:])
        w1t_sb = const.tile([128, 4, 128], BF16)
        for k in range(4):
            nc.sync.dma_start(w1t_sb[:, k, :], w1t_d[k * 128:(k + 1) * 128, :])
        mqkv_sb = const.tile([128, 3, 128], BF16)
        for i in range(3):
            nc.sync.dma_start(mqkv_sb[:, i, :], mqkv_d[i])
        w3t_sb = const.tile([128, 512], BF16)
        nc.sync.dma_start(w3t_sb[:], w3t_d[:])
        bkv_sb = const.tile([128, 2], F32)
        nc.sync.dma_start(bkv_sb[:], bkv_d[:])
        relc_sb = const.tile([128, 49], F32)
        nc.sync.dma_start(relc_sb[:], relc_d[:])
        gb12_sb = const.tile([128, 4], F32)
        nc.sync.dma_start(gb12_sb[:], gb12_d[:])
        gb3_sb = const.tile([128, 8], F32)
        nc.sync.dma_start(gb3_sb[:], gb3_d[:])

        zcol = const.tile([128, 1], F32)
        nc.gpsimd.memset(zcol[:], 0.0)
        expwarm = const.tile([128, 1], F32)
        nc.scalar.activation(expwarm[:], zcol[:], AF.Exp, bias=zcol[:])
        epscol = const.tile([128, 1], F32)
        nc.gpsimd.memset(epscol[:], EPS)

        for _rep in range(_REPS):
            xb_sb = sb.tile([128, 4, HW], BF16)
            for k in range(4):
                for hh in range(2):
                    nc.sync.dma_start(
                        xb_sb[:, k, hh * 512:(hh + 1) * 512],
                        xb_d[k * 128:(k + 1) * 128, hh * 512:(hh + 1) * 512])

            # ---------------- helpers ----------------
            def dump(name, ap):
                if not dbg:
                    return
                n = ap.free_size()
                scr = work.tile([128, 7 * HW], F32, tag="dbgscr", bufs=1,
                                name=f"dbgscr_{name}")[:, 0:n]
                nc.vector.tensor_copy(scr[:], ap)
                nc.sync.dma_start(dbg_d[name][:], scr[:])

            def allreduce(src_ap, ncols, name):
                dst = sb.tile([128, ncols], F32, name=f"cc_{name}_res")
                if _NO_CC:
                    nc.vector.tensor_scalar_mul(dst[:], src_ap, float(N_CORES))
                    return dst
                cin = dram.tile([128, ncols], F32, name=f"cc_{name}_in")
                nc.sync.dma_start(cin[:], src_ap)
                if _CC_MODE == "ag":
                    cout = dram.tile([N_CORES * 128, ncols], F32,
                                     name=f"cc_{name}_out")
                    nc.gpsimd.collective_compute(
                        "AllGather", OP.bypass,
                        replica_groups=[list(range(N_CORES))],
                        ins=[cin[:].opt()], outs=[cout[:].opt()],
                    )
                    gat = sb.tile([128, ncols, N_CORES], F32, name=f"cc_{name}_gat")
                    nc.sync.dma_start(
                        gat[:], cout[:].rearrange("(j p) c -> p c j", p=128))
                    nc.vector.tensor_reduce(dst[:], gat[:], AX.X, OP.add)
                else:
                    cout = dram.tile([128, ncols], F32, name=f"cc_{name}_out")
                    nc.gpsimd.collective_compute(
                        "AllReduce", OP.add,
                        replica_groups=[list(range(N_CORES))],
                        ins=[cin[:].opt()], outs=[cout[:].opt()],
                    )
                    nc.sync.dma_start(dst[:], cout[:])
                return dst

            def bn_params(gsum, gsumsq, gamma, beta, name):
                """a = gamma*rsqrt(var+eps), b = beta - a*mean  (columns [128,1])."""
                mean = sb.tile([128, 1], F32, name=f"bn_{name}_mean")
                msq = sb.tile([128, 1], F32, name=f"bn_{name}_msq")
                nc.scalar.activation(mean[:], gsum, AF.Copy, scale=1.0 / NSAMP)
                nc.scalar.activation(msq[:], gsumsq, AF.Copy, scale=1.0 / NSAMP)
                var = sb.tile([128, 1], F32, name=f"bn_{name}_var")
                nc.vector.tensor_tensor(var[:], mean[:], mean[:], OP.mult)
                nc.vector.tensor_tensor(var[:], msq[:], var[:], OP.subtract)
                lg = sb.tile([128, 1], F32, name=f"bn_{name}_lg")
                nc.scalar.activation(lg[:], var[:], AF.Ln, bias=epscol[:])
                rsd = sb.tile([128, 1], F32, name=f"bn_{name}_rsd")
                nc.scalar.activation(rsd[:], lg[:], AF.Exp, bias=zcol[:], scale=-0.5)
                a = sb.tile([128, 1], F32, name=f"bn_{name}_a")
                b = sb.tile([128, 1], F32, name=f"bn_{name}_b")
                nc.vector.tensor_tensor(a[:], gamma, rsd[:], OP.mult)
                nc.vector.tensor_tensor(b[:], a[:], mean[:], OP.mult)
                nc.vector.tensor_tensor(b[:], beta, b[:], OP.subtract)
                return a, b

            # ---------------- conv1 (512 -> 128) ----------------
            o1_ps = psum.tile([128, HW], F32, tag="A")
            for n in range(2):
                for k in range(4):
                    nc.tensor.matmul(
                        o1_ps[:, n * 512:(n + 1) * 512],
                        w1t_sb[:, k, :],
                        xb_sb[:, k, n * 512:(n + 1) * 512],
                        start=(k == 0), stop=(k == 3),
                    )

            s1 = sb.tile([128, 2], F32)
            nc.vector.tensor_reduce(s1[:, 0:1], o1_ps[:], AX.X, OP.add)
            sq1 = work.tile([128, HW], BF16, tag="sq", bufs=1)
            nc.scalar.activation(sq1[:], o1_ps[:], AF.Square, bias=zcol[:],
                                 accum_out=s1[:, 1:2])
            dump("o1", o1_ps[:])
            g1 = allreduce(s1[:], 2, "bn1")
            a1, b1 = bn_params(g1[:, 0:1], g1[:, 1:2],
                               gb12_sb[:, 0:1], gb12_sb[:, 1:2], "1")

            # x1 = relu(a1*o1 + b1), written into the zero-padded 38x38 field
            x1p = sb.tile([128, PW, PW], BF16)
            nc.gpsimd.memset(x1p[:], 0.0)
            nc.scalar.activation(
                x1p[:, PAD:PAD + H, PAD:PAD + W],
                o1_ps[:].rearrange("p (y x) -> p y x", y=H),
                AF.Relu, bias=b1[:], scale=a1[:],
            )

            dump("x1p", x1p[:].rearrange("p y x -> p (y x)"))
            # ---------------- q/k/v grouped 1x1 convs ----------------

            col_splits = [(0, 512), (512, 1024), (1024, PHW)]

            def kv_conv(widx, bias_col, name, ps_tag):
                ps = psum.tile([128, PHW], F32, tag=ps_tag, name=f"{name}_ps")
                x1p_flat = x1p[:].rearrange("p y x -> p (y x)")
                for (c0, c1) in col_splits:
                    nc.tensor.matmul(
                        ps[:, c0:c1],
                        mqkv_sb[:, widx, :],
                        x1p_flat[:, c0:c1],
                        start=True, stop=True,
                    )
                fld = sb.tile([128, PW, PW], BF16, name=f"{name}_fld")
                nc.scalar.activation(
                    fld[:], ps[:].rearrange("p (y x) -> p y x", y=PW),
                    AF.Identity, bias=bias_col,
                )
                odd = sb.tile([128, PHW - 1], BF16, name=f"{name}_odd")
                nc.vector.tensor_copy(
                    odd[:], fld[:].rearrange("p y x -> p (y x)")[:, 1:PHW])
                return fld, odd

            q_ps = psum.tile([128, HW], F32, tag="A")
            for n in range(2):
                nc.tensor.matmul(
                    q_ps[:, n * 512:(n + 1) * 512],
                    mqkv_sb[:, 0, :],
                    x1p[:, PAD + n * 16:PAD + (n + 1) * 16, PAD:PAD + W],
                    start=True, stop=True,
                )
            q_bf = sb.tile([128, HW], BF16)
            nc.scalar.activation(q_bf[:], q_ps[:], AF.Copy)
            dump("q", q_bf[:])
            k_fld, k_odd = kv_conv(1, bkv_sb[:, 0:1], "k", "B")
            dump("kf", k_fld[:].rearrange("p y x -> p (y x)"))
            v_fld, v_odd = kv_conv(2, bkv_sb[:, 1:2], "v", "C")

            # ---------------- attention over 49 shifts ----------------
            z_ps = psum.tile([128, HW], F32, tag="A")
            s_ps = psum.tile([128, HW], F32, tag="B")

            STT_KWS = {6, 5}     # fused (k+rel)*q planes (last of each parity block)
            for kh in range([0, KS][not _NO_ATT]):
                mb = work.tile([128, KS, HW], BF16, tag="mb")
                q3 = q_bf[:].rearrange("p (y x) -> p y x", y=H)
                # (k_shift + rel) per plane; rel is a per-partition scalar
                for pos, kw in enumerate(KW_ORDER):
                    kap = kh * KS + kw
                    if kw % 2 == 0:
                        view = k_fld[:, kh:kh + H, kw:kw + W]
                    else:
                        view = _sview(k_odd[:], kh * PW + kw - 1,
                                      [(PW, H), (1, W)])
                    dst = mb[:, pos, :].rearrange("p (y x) -> p y x", y=H)
                    if kw in STT_KWS:
                        nc.vector.scalar_tensor_tensor(
                            dst, view, relc_sb[:, kap:kap + 1], q3,
                            OP.add, OP.mult)
                    elif kw in ACT_ADD_KWS:
                        nc.scalar.activation(dst, view, AF.Identity,
                                             bias=relc_sb[:, kap:kap + 1])
                    elif kw in POOL_ADD_KWS:
                        nc.gpsimd.tensor_scalar_add(dst, view,
                                                    relc_sb[:, kap:kap + 1])
                    else:
                        nc.vector.tensor_scalar_add(dst, view,
                                                    relc_sb[:, kap:kap + 1])
                # l = m * q  (parity-batched, bf16 2x, in place over mb;
                # planes 3 (kw6) and 6 (kw5) already hold l via the fused op)
                nc.vector.tensor_tensor(
                    mb[:, 0:3, :], mb[:, 0:3, :],
                    q_bf[:].rearrange("p (a h) -> p a h", a=1).to_broadcast([128, 3, HW]),
                    OP.mult)
                nc.vector.tensor_tensor(
                    mb[:, 4:6, :], mb[:, 4:6, :],
                    q_bf[:].rearrange("p (a h) -> p a h", a=1).to_broadcast([128, 2, HW]),
                    OP.mult)
                # e = exp(l)
                eb = work.tile([128, KS, HW], BF16, tag="eb", bufs=3)
                nc.scalar.activation(eb[:], mb[:], AF.Exp, bias=zcol[:])
                # Z += sum_planes(e) on the TensorEngine
                for pl in range(KS):
                    for hh in range(2):
                        nc.tensor.matmul(
                            z_ps[:, hh * 512:(hh + 1) * 512],
                            id_sb[:], eb[:, pl, hh * 512:(hh + 1) * 512],
                            start=(kh == 0 and pl == 0), stop=(kh == KS - 1 and pl == KS - 1),
                            skip_group_check=True,
                        )
                if kh == 0:
                    dump("eb0", eb[:].rearrange("p k h -> p (k h)"))
                # ev = e * v_shift (parity-batched)
                evb = work.tile([128, KS, HW], BF16, tag="evb")
                v_flat = v_fld[:].rearrange("p y x -> p (y x)")
                ev_even = _sview(v_flat, kh * PW, [(2, 4), (PW, H), (1, W)])
                ev_odd = _sview(v_odd[:], kh * PW, [(2, 3), (PW, H), (1, W)])
                nc.vector.tensor_tensor(
                    evb[:, 0:4, :].rearrange("p k (y x) -> p k y x", y=H),
                    eb[:, 0:4, :].rearrange("p k (y x) -> p k y x", y=H),
                    ev_even, OP.mult)
                nc.vector.tensor_tensor(
                    evb[:, 4:7, :].rearrange("p k (y x) -> p k y x", y=H),
                    eb[:, 4:7, :].rearrange("p k (y x) -> p k y x", y=H),
                    ev_odd, OP.mult)
                # S += sum_planes(ev)
                for pl in range(KS):
                    for hh in range(2):
                        nc.tensor.matmul(
                            s_ps[:, hh * 512:(hh + 1) * 512],
                            id_sb[:], evb[:, pl, hh * 512:(hh + 1) * 512],
                            start=(kh == 0 and pl == 0), stop=(kh == KS - 1 and pl == KS - 1),
                            skip_group_check=True,
                        )

            # att = S / Z, then BN2 + relu
            dump("z", z_ps[:])
            dump("s", s_ps[:])
            rz = sb.tile([128, HW], F32)
            nc.vector.reciprocal_approx_fast(rz[:], z_ps[:])
            att = sb.tile([128, HW], F32)
            nc.vector.tensor_tensor(att[:], s_ps[:], rz[:], OP.mult)

            dump("att", att[:])
            s2 = sb.tile([128, 2], F32)
            nc.vector.tensor_reduce(s2[:, 0:1], att[:], AX.X, OP.add)
            sq2 = work.tile([128, HW], BF16, tag="sq", bufs=1)
            nc.scalar.activation(sq2[:], att[:], AF.Square, bias=zcol[:],
                                 accum_out=s2[:, 1:2])
            g2 = allreduce(s2[:], 2, "bn2")
            a2, b2 = bn_params(g2[:, 0:1], g2[:, 1:2],
                               gb12_sb[:, 2:3], gb12_sb[:, 3:4], "2")
            x2 = sb.tile([128, HW], BF16)
            nc.scalar.activation(x2[:], att[:], AF.Relu, bias=b2[:], scale=a2[:])

            # ---------------- conv3 (128 -> 512) + BN3 + residual ----------------
            xf_sb = sb.tile([128, 4, HW], F32)
            for k in range(4):
                nc.sync.dma_start(xf_sb[:, k, :], xf_d[k * 128:(k + 1) * 128, :])
            o3_sb = sb.tile([128, 4, HW], F32)
            s3 = sb.tile([128, 8], F32)
            for j in range(4):
                o3_ps = psum.tile([128, HW], F32, tag=["C", "A"][j % 2], name=f"o3_ps_{j}")
                for n in range(2):
                    nc.tensor.matmul(
                        o3_ps[:, n * 512:(n + 1) * 512],
                        w3t_sb[:, j * 128:(j + 1) * 128],
                        x2[:, n * 512:(n + 1) * 512],
                        start=True, stop=True,
                    )
                sq3 = work.tile([128, HW], BF16, tag="sq", bufs=1, name=f"sq3_{j}")
                nc.scalar.activation(sq3[:], o3_ps[:], AF.Square, bias=zcol[:],
                                     accum_out=s3[:, 2 * j + 1:2 * j + 2])
                nc.scalar.activation(o3_sb[:, j, :], o3_ps[:], AF.Copy,
                                     accum_out=s3[:, 2 * j:2 * j + 1])

            dump("o3", o3_sb[:].rearrange("p j h -> p (j h)"))
            g3 = allreduce(s3[:], 8, "bn3")
            # batched BN3 params for all 4 chunks: [128, 4] columns
            sc3 = sb.tile([128, 8], F32)
            nc.scalar.activation(sc3[:], g3[:], AF.Copy, scale=1.0 / NSAMP)
            mean3 = sc3[:, 0:8:2]
            msq3 = sc3[:, 1:8:2]
            m23 = sb.tile([128, 4], F32)
            nc.vector.tensor_tensor(m23[:], mean3, mean3, OP.mult)
            var3 = sb.tile([128, 4], F32)
            nc.vector.tensor_tensor(var3[:], msq3, m23[:], OP.subtract)
            lg3 = sb.tile([128, 4], F32)
            nc.scalar.activation(lg3[:], var3[:], AF.Ln, bias=epscol[:])
            rsd3 = sb.tile([128, 4], F32)
            nc.scalar.activation(rsd3[:], lg3[:], AF.Exp, bias=zcol[:], scale=-0.5)
            a3 = sb.tile([128, 4], F32)
            b3 = sb.tile([128, 4], F32)
            nc.vector.tensor_tensor(a3[:], gb3_sb[:, 0:8:2], rsd3[:], OP.mult)
            nc.vector.tensor_tensor(b3[:], a3[:], mean3, OP.mult)
            nc.vector.tensor_tensor(b3[:], gb3_sb[:, 1:8:2], b3[:], OP.subtract)
            for j in range(4):
                t3 = work.tile([128, HW], F32, tag="t3", name=f"t3_{j}")
                nc.vector.scalar_tensor_tensor(t3[:], o3_sb[:, j, :], a3[:, j:j + 1],
                                               xf_sb[:, j, :], OP.mult, OP.add)
                ot = work.tile([128, HW], F32, tag="ot", name=f"ot_{j}")
                nc.scalar.activation(ot[:], t3[:], AF.Relu, bias=b3[:, j:j + 1])
                nc.sync.dma_start(out_d[j * 128:(j + 1) * 128, :], ot[:])

    nc.compile()
    return nc


_NC = None


def _get_nc():
    global _NC
    if _NC is None:
        _NC = _build_nc()
    return _NC


def _prep_inputs(x, W1, g1, b1, Wq, Wk, bk, Wv, bv, rel_x, rel_y, g2, b2, W3, g3, b3):
    f32 = np.float32
    bf = ml_dtypes.bfloat16

    # channel permutation: new partition -> old channel within the 128 planes
    perm = np.zeros(PLANES, dtype=np.int64)
    for g in range(GROUPS):
        for d in range(D):
            p = g * REL + d if d < REL else 64 + g * REL + (d - REL)
            perm[p] = g * D + d

    W1p = np.ascontiguousarray(W1[perm, :])                     # [128, 512]
    w1t = np.ascontiguousarray(W1p.T).astype(bf)                # [512, 128]

    def block_mat(Wg):
        M = np.zeros((PLANES, PLANES), dtype=f32)
        for po in range(PLANES):
            g = (po % 64) // REL
            o = perm[po] - g * D
            for pi_d in range(D):
                pi = g * REL + pi_d if pi_d < REL else 64 + g * REL + (pi_d - REL)
                M[po, pi] = Wg[g, o, pi_d]
        return M

    mqkv = np.stack([np.ascontiguousarray(block_mat(Wg).T)
                     for Wg in (Wq, Wk, Wv)]).astype(bf)        # [3,128,128] (lhsT)

    bkv = np.stack([bk.reshape(-1)[perm], bv.reshape(-1)[perm]], axis=1).astype(f32)

    # rel columns [128, 49]
    relc = np.zeros((PLANES, KS * KS), dtype=f32)
    for p in range(PLANES):
        g = (p % 64) // REL
        dd = perm[p] - g * D
        for kap in range(KS * KS):
            kh, kw = divmod(kap, KS)
            relc[p, kap] = rel_x[dd, kh, 0] if dd < REL else rel_y[dd - REL, 0, kw]

    gb12 = np.stack([g1[perm], b1[perm], g2[perm], b2[perm]], axis=1).astype(f32)

    W3p = np.ascontiguousarray(W3[:, perm])                     # [512, 128]
    w3t = np.ascontiguousarray(W3p.T).astype(bf)                # [128, 512]

    gb3 = np.zeros((PLANES, 8), dtype=f32)
    for j in range(4):
        gb3[:, 2 * j] = g3[j * 128:(j + 1) * 128]
        gb3[:, 2 * j + 1] = b3[j * 128:(j + 1) * 128]

    id128 = np.eye(PLANES, dtype=f32).astype(bf)

    shared = dict(w1t=w1t, mqkv=mqkv, w3t=w3t, bkv=bkv, relc=relc,
                  gb12=gb12, gb3=gb3, id128=id128)
    in_maps = []
    for c in range(N_CORES):
        xi = np.ascontiguousarray(x[c].reshape(C_IN, HW)).astype(f32)
        m = dict(shared)
        m["xf"] = xi
        m["xb"] = xi.astype(bf)
        in_maps.append(m)
    return in_maps


def _run(inputs, **kw):
    nc = _get_nc()
    in_maps = _prep_inputs(**inputs)
    res = run_bass_kernel_spmd(nc, in_maps, core_ids=list(range(N_CORES)), **kw)
    out = np.stack([res.results[c]["out"].reshape(C_IN, H, W)
                    for c in range(N_CORES)]).astype(np.float32)
    return out, res


def kernel(**inputs):
    out, _ = _run(inputs)
    return out



# revision 28
# speedup vs baseline: 1.5293x; 1.5293x over previous
# Trainium2 Bass kernel for nn_BottleNeck (sparse local attention bottleneck).
#
# Sharding: data-parallel over batch (B=8 -> 8 cores, one image each).
# BatchNorm batch statistics are combined with three tiny AllGather
# collectives whose latency is software-pipelined against compute.
#
# On-chip layout: channels on partitions, hw=32*32=1024 on the free dim.
# Channels are PERMUTED so partitions 0:63 hold the "x-type" attention
# channels (rel depends only on kh) and 64:127 the "y-type" (rel depends
# only on kw); the permutation is folded into the weights host-side.
#
# Attention restructure vs the naive 49-shift loop:
#  * y-type channels store their k/v/q fields TRANSPOSED (rows<->cols).
#    With outer step j and inner plane i the x-half computes shift
#    (kh=j, kw=i) while the y-half computes (kh=i, kw=j) -- both read
#    field_j at view [rows 0:32, cols i:i+32], so one AP spans all 128
#    partitions, and rel (x: f(kh)=f(j); y: f(kw)=f(j)) is a per-
#    partition constant folded into field_j ONCE (7 field builds replace
#    49 per-plane rel adds).  The y-half's att output comes out pixel-
#    transposed and is untransposed for free in the BN2 relu's input AP.
#  * qk and e*v multiplies run as parity-batched bf16 2x tensor_tensor
#    on DVE (odd-i planes read one-element-shifted field copies so
#    packed reads stay 4B aligned); 2 of 7 e*v planes go to Pool.
#  * exp runs on ACT; all ACT functions used stay inside one activation
#    table (BN rsqrt is Newton iteration on DVE/Pool, not Ln/Exp), so
#    the table never reloads.
#  * Z = sum(e), S = sum(e*v) accumulate on the TensorEngine as identity
#    matmuls into PSUM.
#  * The j-loop is software-pipelined (DVE computes qk(j+1) before
#    ev(j)) so ACT exps run back to back.
# Cross-rep schedule (steady state): conv1/stats1/AR1 of rep i+1 run in
# the middle of attention(i) on PE/Pool; AR2(i) is issued right after
# attention(i) and its latency is covered by stageB(i+1) (x1p, q/k/v
# convs, field builds), so exp(i+1, 0) starts before AR2(i) completes;
# conv3(i), stats3 (Pool), AR3(i), bn3 and the residual+relu+output
# DMAs of rep i all ride inside attention(i+1).

import os
from contextlib import ExitStack

import numpy as np
import ml_dtypes

import concourse.bass as bass
import concourse.mybir as mybir
import concourse.tile as tile
from concourse import bacc
from concourse.ap import AP
from concourse.bass_utils import run_bass_kernel_spmd

F32 = mybir.dt.float32
I32 = mybir.dt.int32
BF16 = mybir.dt.bfloat16
AF = mybir.ActivationFunctionType
OP = mybir.AluOpType
AX = mybir.AxisListType

B, C_IN, H, W = 8, 512, 32, 32
PLANES, GROUPS, KS, PAD = 128, 8, 7, 3
D = PLANES // GROUPS
REL = D // 2
HW = H * W
PW = W + 2 * PAD            # 38
PHW = PW * PW               # 1444
FW = H * PW                 # 1216: one j-field (32 rows x 38 cols)
EPS = 1e-5
N_CORES = 8
NSAMP = float(B * HW)
MAGIC = 0x5F3759DF

_NO_CC = os.environ.get("BASS_NO_CC") == "1"
_CC_MODE = os.environ.get("BASS_CC_MODE", "ag")    # ag=AllGather+sum, ar=AllReduce
_REPS = int(os.environ.get("BASS_REPS", "1"))
# which of the 7 e*v planes run on Pool (plane idx 0-3=even i, 4-6=odd i;
# must leave a contiguous prefix of each parity run on DVE)
_pev = os.environ.get("BASS_POOL_EV", "")
POOL_EV = [int(v) for v in _pev.split(",") if v != ""]
# heavy Pool offloads (conv3 copies/stats, residual stt, o1 stats) -- set
# BASS_POOL_LIGHT=1 to move them back to ACT/DVE if HW Pool is slow.
_POOL_LIGHT = os.environ.get("BASS_POOL_LIGHT") == "1"


def _sv(flat_ap, off, dims):
    """Strided view of a (possibly partition-sliced) flat AP."""
    return AP(flat_ap.tensor, flat_ap.offset + off,
              [list(flat_ap.ap[0])] + [list(d) for d in dims])


def _build_nc():
    nc = bacc.Bacc("TRN2", target_bir_lowering=False, debug=False,
                   num_devices=N_CORES)

    xb_d = nc.dram_tensor("xb", [C_IN, HW], BF16, kind="ExternalInput")
    w1t_d = nc.dram_tensor("w1t", [C_IN, PLANES], BF16, kind="ExternalInput")
    mqkv_d = nc.dram_tensor("mqkv", [3, PLANES, PLANES], BF16, kind="ExternalInput")
    w3t_d = nc.dram_tensor("w3t", [PLANES, 4 * PLANES], BF16, kind="ExternalInput")
    bkv_d = nc.dram_tensor("bkv", [PLANES, 2], F32, kind="ExternalInput")
    relj_d = nc.dram_tensor("relj", [PLANES, KS], F32, kind="ExternalInput")
    gb12_d = nc.dram_tensor("gb12", [PLANES, 4], F32, kind="ExternalInput")
    gb3_d = nc.dram_tensor("gb3", [PLANES, 8], F32, kind="ExternalInput")
    id_d = nc.dram_tensor("id128", [PLANES, PLANES], BF16, kind="ExternalInput")
    out_d = nc.dram_tensor("out", [C_IN, HW], F32, kind="ExternalOutput")

    with tile.TileContext(nc) as tc, ExitStack() as ctx:
        const = ctx.enter_context(tc.tile_pool(name="const", bufs=1))
        sb = ctx.enter_context(tc.tile_pool(name="sb", bufs=1))
        sb3 = ctx.enter_context(tc.tile_pool(name="sb3", bufs=3))
        work = ctx.enter_context(tc.tile_pool(name="work", bufs=2))
        psum = ctx.enter_context(tc.tile_pool(name="psum", bufs=1, space="PSUM"))
        dram = ctx.enter_context(tc.tile_pool(name="dram", bufs=1, space="DRAM"))

        # ---------------- constants / weights ----------------
        id_sb = const.tile([128, 128], BF16)
        nc.sync.dma_start(id_sb[:], id_d[:])
        w1t_sb = const.tile([128, 4, 128], BF16)
        for k in range(4):
            nc.sync.dma_start(w1t_sb[:, k, :], w1t_d[k * 128:(k + 1) * 128, :])
        mqkv_sb = const.tile([128, 3, 128], BF16)
        for i in range(3):
            nc.sync.dma_start(mqkv_sb[:, i, :], mqkv_d[i])
        w3t_sb = const.tile([128, 512], BF16)
        nc.sync.dma_start(w3t_sb[:], w3t_d[:])
        bkv_sb = const.tile([128, 2], F32)
        nc.sync.dma_start(bkv_sb[:], bkv_d[:])
        relj_sb = const.tile([128, KS], F32)
        nc.sync.dma_start(relj_sb[:], relj_d[:])
        gb12_sb = const.tile([128, 4], F32)
        nc.sync.dma_start(gb12_sb[:], gb12_d[:])
        gb3_sb = const.tile([128, 8], F32)
        nc.sync.dma_start(gb3_sb[:], gb3_d[:])

        zcol = const.tile([128, 1], F32)
        nc.gpsimd.memset(zcol[:], 0.0)
        epscol = const.tile([128, 1], F32)
        nc.gpsimd.memset(epscol[:], EPS)
        warm = const.tile([128, 1], F32)
        nc.scalar.activation(warm[:], zcol[:], AF.Exp, bias=zcol[:])

        # padded BN1 field; border is zero forever, memset once.
        x1p = const.tile([128, PW, PW], BF16)
        nc.gpsimd.memset(x1p[:], 0.0)

        # ---------------- helpers ----------------
        def ar_issue(src_ap, ncols, name):
            if _NO_CC:
                return ("nocc", src_ap, ncols, name)
            cin = dram.tile([128, ncols], F32, name=f"cc_{name}_in")
            nc.sync.dma_start(cin[:], src_ap)
            if _CC_MODE == "ag":
                cout = dram.tile([N_CORES * 128, ncols], F32, name=f"cc_{name}_out")
                nc.gpsimd.collective_compute(
                    "AllGather", OP.bypass,
                    replica_groups=[list(range(N_CORES))],
                    ins=[cin[:].opt()], outs=[cout[:].opt()],
                )
            else:
                cout = dram.tile([128, ncols], F32, name=f"cc_{name}_out")
                nc.gpsimd.collective_compute(
                    "AllReduce", OP.add,
                    replica_groups=[list(range(N_CORES))],
                    ins=[cin[:].opt()], outs=[cout[:].opt()],
                )
            return ("cc", cout, ncols, name)

        def ar_wait(handle, eng):
            mode, cout, ncols, name = handle
            dst = sb.tile([128, ncols], F32, name=f"cc_{name}_res", tag=f"cc_{name}_res")
            if mode == "nocc":
                eng.tensor_scalar_mul(dst[:], cout, float(N_CORES))
                return dst
            if _CC_MODE == "ag":
                gat = sb.tile([128, ncols, N_CORES], F32, name=f"cc_{name}_gat",
                              tag=f"cc_{name}_gat")
                nc.sync.dma_start(
                    gat[:], cout[:].rearrange("(j p) c -> p c j", p=128))
                # pairwise 8-way sum (works on DVE and Pool alike)
                t4 = sb.tile([128, ncols, 4], F32, name=f"cc_{name}_t4",
                             tag=f"cc_{name}_t4")
                eng.tensor_tensor(t4[:], gat[:, :, 0:4], gat[:, :, 4:8], OP.add)
                eng.tensor_tensor(t4[:, :, 0:2], t4[:, :, 0:2], t4[:, :, 2:4],
                                  OP.add)
                eng.tensor_tensor(dst[:], t4[:, :, 0], t4[:, :, 1], OP.add)
            else:
                nc.sync.dma_start(dst[:], cout[:])
            return dst

        def bn_params(eng, g, gamma, beta, name, n=1):
            """a = gamma*rsqrt(var+eps), b = beta - a*mean (baseline-proven
            Ln/Exp path; eng unused). g: [128, 2n] interleaved (sum, sumsq)."""
            sc = sb.tile([128, 2 * n], F32, name=f"bn_{name}_sc", tag=f"bn_{name}_sc")
            nc.scalar.activation(sc[:], g[:], AF.Copy, scale=1.0 / NSAMP)
            mean = sc[:, 0:2 * n:2]
            msq = sc[:, 1:2 * n:2]
            m2 = sb.tile([128, n], F32, name=f"bn_{name}_m2", tag=f"bn_{name}_m2")
            nc.vector.tensor_tensor(m2[:], mean, mean, OP.mult)
            var = sb.tile([128, n], F32, name=f"bn_{name}_var", tag=f"bn_{name}_var")
            nc.vector.tensor_tensor(var[:], msq, m2[:], OP.subtract)
            lg = sb.tile([128, n], F32, name=f"bn_{name}_lg", tag=f"bn_{name}_lg")
            nc.scalar.activation(lg[:], var[:], AF.Ln, bias=epscol[:])
            rsd = sb.tile([128, n], F32, name=f"bn_{name}_rsd", tag=f"bn_{name}_rsd")
            nc.scalar.activation(rsd[:], lg[:], AF.Exp, bias=zcol[:], scale=-0.5)
            a = sb.tile([128, n], F32, name=f"bn_{name}_a", tag=f"bn_{name}_a")
            b = sb.tile([128, n], F32, name=f"bn_{name}_b", tag=f"bn_{name}_b")
            nc.vector.tensor_tensor(a[:], gamma, rsd[:], OP.mult)
            nc.vector.tensor_tensor(b[:], a[:], mean, OP.mult)
            nc.vector.tensor_tensor(b[:], beta, b[:], OP.subtract)
            return a, b

        # ---------------- per-rep stages ----------------
        def xb_dma(i):
            xb_sb = sb3.tile([128, 4, HW], BF16, tag="xb", name=f"xb_{i}")
            for k in range(4):
                for hh in range(2):
                    nc.sync.dma_start(
                        xb_sb[:, k, hh * 512:(hh + 1) * 512],
                        xb_d[k * 128:(k + 1) * 128, hh * 512:(hh + 1) * 512])
            return xb_sb

        def stageA_conv(i, xb_sb):
            """conv1 + stats1 (PE + Pool only). Returns (o1b, s1)."""
            o1_ps = psum.tile([128, HW], F32, tag="TW", name=f"o1_ps_{i}")
            for n in range(2):
                for k in range(4):
                    nc.tensor.matmul(
                        o1_ps[:, n * 512:(n + 1) * 512],
                        w1t_sb[:, k, :],
                        xb_sb[:, k, n * 512:(n + 1) * 512],
                        start=(k == 0), stop=(k == 3),
                    )
            s1 = sb.tile([128, 2], F32, tag="s1", name=f"s1_{i}")
            nc.vector.tensor_reduce(s1[:, 0:1], o1_ps[:], AX.X, OP.add)
            scr = work.tile([128, HW], BF16, tag="scr", bufs=1,
                            name=f"sq1_{i}")
            nc.scalar.activation(scr[:], o1_ps[:], AF.Square, bias=zcol[:],
                                 accum_out=s1[:, 1:2])
            return o1_ps, s1

        def stageB1(i, o1_ps, ab1):
            """x1p relu, k conv + field copies, q conv + copies.
            Emitted before the final Z/S drain so PE serves these first."""
            a1, b1 = ab1
            nc.scalar.activation(
                x1p[:, PAD:PAD + H, PAD:PAD + W],
                o1_ps[:].rearrange("p (y x) -> p y x", y=H),
                AF.Relu, bias=b1[:], scale=a1[:],
            )
            x1p_flat = x1p[:].rearrange("p y x -> p (y x)")
            col_splits = [(0, 512), (512, 1024), (1024, PHW)]

            # ---- k conv + mixed-layout field (y-half transposed) ----
            k_ps = psum.tile([128, PHW], F32, tag="TW", name=f"k_ps_{i}")
            for (c0, c1) in col_splits:
                nc.tensor.matmul(k_ps[:, c0:c1], mqkv_sb[:, 1, :],
                                 x1p_flat[:, c0:c1], start=True, stop=True)
            kfld = sb.tile([128, PHW], BF16, tag="kfld", name=f"kfld_{i}")
            nc.scalar.activation(
                kfld[64:128].rearrange("p (u w) -> p u w", u=PW),
                _sv(k_ps[64:128], 0, [(1, PW), (PW, PW)]),
                AF.Identity, bias=bkv_sb[64:128, 0:1])
            nc.scalar.activation(
                kfld[0:64].rearrange("p (y x) -> p y x", y=PW),
                k_ps[0:64].rearrange("p (y x) -> p y x", y=PW),
                AF.Identity, bias=bkv_sb[0:64, 0:1])

            # ---- q conv (1-bank psum, 2 passes) + copies (y transposed) ----
            q_bf = sb.tile([128, HW], BF16, tag="q_bf", name=f"q_bf_{i}")
            for n in range(2):
                q_ps = psum.tile([128, 512], F32, tag="TQ", name=f"q_ps_{i}_{n}")
                nc.tensor.matmul(
                    q_ps[:],
                    mqkv_sb[:, 0, :],
                    x1p[:, PAD + n * 16:PAD + (n + 1) * 16, PAD:PAD + W],
                    start=True, stop=True)
                nc.vector.tensor_copy(q_bf[0:64, n * 512:(n + 1) * 512],
                                      q_ps[0:64])
                nc.vector.tensor_copy(
                    _sv(q_bf[64:128], n * 16, [(1, 16), (H, H)]),
                    q_ps[64:128].rearrange("p (y x) -> p y x", y=16))
            return dict(q=q_bf, kfld=kfld, x1p_flat=x1p_flat,
                        col_splits=col_splits)

        def stageB2(i, bi):
            """kfldo, per-j field 0, v conv + fields. Emitted after the
            final Z/S drain (DVE ev(6) precedes kfldo in stream order)."""
            kfld = bi.pop("kfld")
            x1p_flat = bi.pop("x1p_flat")
            col_splits = bi.pop("col_splits")
            kfldo = sb.tile([128, PHW], BF16, tag="kfldo", name=f"kfldo_{i}")
            nc.vector.tensor_copy(kfldo[:, 0:PHW - 1], kfld[:, 1:PHW])
            nc.gpsimd.memset(kfldo[:, PHW - 1:PHW], 0.0)

            kf2 = sb.tile([128, KS, FW], BF16, tag="kf2", name=f"kf2_{i}")
            kf2o = sb.tile([128, KS, FW], BF16, tag="kf2o", name=f"kf2o_{i}")

            def build_j(j):
                nc.vector.tensor_scalar_add(
                    kf2[:, j, :], _sv(kfld[:], j * PW, [(1, FW)]),
                    relj_sb[:, j:j + 1])
                nc.vector.tensor_scalar_add(
                    kf2o[:, j, :], _sv(kfldo[:], j * PW, [(1, FW)]),
                    relj_sb[:, j:j + 1])

            build_j(0)

            # ---- v conv + fields ----
            v_ps = psum.tile([128, PHW], F32, tag="TW", name=f"v_ps_{i}")
            for (c0, c1) in col_splits:
                nc.tensor.matmul(v_ps[:, c0:c1], mqkv_sb[:, 2, :],
                                 x1p_flat[:, c0:c1], start=True, stop=True)
            vf = sb.tile([128, PHW], BF16, tag="vf", name=f"vf_{i}")
            nc.scalar.activation(
                vf[64:128].rearrange("p (u w) -> p u w", u=PW),
                _sv(v_ps[64:128], 0, [(1, PW), (PW, PW)]),
                AF.Identity, bias=bkv_sb[64:128, 1:2])
            nc.vector.tensor_scalar_add(
                vf[0:64], v_ps[0:64], bkv_sb[0:64, 1:2])
            vfo = sb.tile([128, PHW], BF16, tag="vfo", name=f"vfo_{i}")
            nc.vector.tensor_copy(vfo[:, 0:PHW - 1], vf[:, 1:PHW])
            nc.gpsimd.memset(vfo[:, PHW - 1:PHW], 0.0)
            bi.update(kf2=kf2, kf2o=kf2o, vf=vf, vfo=vfo, build_j=build_j)
            return bi

        def att_mults(i, j, bi, st):
            mb = work.tile([128, KS, HW], BF16, tag="mb", name=f"mb_{i}_{j}")
            kf2f = bi["kf2"][:].rearrange("p a b -> p (a b)")
            kf2of = bi["kf2o"][:].rearrange("p a b -> p (a b)")
            qf = bi["q"][:]
            nc.vector.tensor_tensor(
                _sv(mb[:], 0, [(HW, 4), (W, H), (1, W)]),
                _sv(kf2f, j * FW, [(2, 4), (PW, H), (1, W)]),
                _sv(qf, 0, [(0, 4), (W, H), (1, W)]),
                OP.mult)
            nc.vector.tensor_tensor(
                _sv(mb[:], 4 * HW, [(HW, 3), (W, H), (1, W)]),
                _sv(kf2of, j * FW, [(2, 3), (PW, H), (1, W)]),
                _sv(qf, 0, [(0, 3), (W, H), (1, W)]),
                OP.mult)
            eb = work.tile([128, KS, HW], BF16, tag="eb", name=f"eb_{i}_{j}")
            nc.scalar.activation(eb[:, 0:4], mb[:, 0:4], AF.Exp, bias=zcol[:])
            nc.scalar.activation(eb[:, 4:7], mb[:, 4:7], AF.Exp, bias=zcol[:])
            st["eb"][j] = eb

        def att_zmm(i, j, st, z_ps):
            eb = st["eb"][j]
            for pl in range(KS):
                for hh in range(2):
                    nc.tensor.matmul(
                        z_ps[:, hh * 512:(hh + 1) * 512],
                        id_sb[:], eb[:, pl, hh * 512:(hh + 1) * 512],
                        start=(j == 0 and pl == 0),
                        stop=(j == KS - 1 and pl == KS - 1),
                        skip_group_check=True)

        def att_ev(i, j, bi, st):
            eb = st["eb"][j]
            evb = work.tile([128, KS, HW], BF16, tag="evb", name=f"evb_{i}_{j}")
            vff = bi["vf"][:]
            vfof = bi["vfo"][:]
            pl_src = {0: (vff, 0), 1: (vff, 2), 2: (vff, 4), 3: (vff, 6),
                      4: (vfof, 0), 5: (vfof, 2), 6: (vfof, 4)}
            dve_even = [p for p in (0, 1, 2, 3) if p not in POOL_EV]
            dve_odd = [p for p in (4, 5, 6) if p not in POOL_EV]
            assert dve_even == list(range(len(dve_even)))
            assert dve_odd == list(range(4, 4 + len(dve_odd)))
            ebf = eb[:].rearrange("p a b -> p (a b)")
            for planes in (dve_even, dve_odd):
                if planes:
                    p0, n = planes[0], len(planes)
                    src, c0 = pl_src[p0]
                    nc.vector.tensor_tensor(
                        _sv(evb[:], p0 * HW, [(HW, n), (W, H), (1, W)]),
                        _sv(ebf, p0 * HW, [(HW, n), (W, H), (1, W)]),
                        _sv(src, j * PW + c0, [(2, n), (PW, H), (1, W)]),
                        OP.mult)
            for p in POOL_EV:
                src, c0 = pl_src[p]
                nc.gpsimd.tensor_tensor(
                    _sv(evb[:], p * HW, [(W, H), (1, W)]),
                    _sv(ebf, p * HW, [(W, H), (1, W)]),
                    _sv(src, j * PW + c0, [(PW, H), (1, W)]),
                    OP.mult)
            st["evb"][j] = evb

        def att_smm(i, j, st, s_ps):
            evb = st["evb"][j]
            for pl in range(KS):
                for hh in range(2):
                    nc.tensor.matmul(
                        s_ps[:, hh * 512:(hh + 1) * 512],
                        id_sb[:], evb[:, pl, hh * 512:(hh + 1) * 512],
                        start=(j == 0 and pl == 0),
                        stop=(j == KS - 1 and pl == KS - 1),
                        skip_group_check=True)

        def stageC(i, z_ps, s_ps):
            """S/Z -> att, stats2, AR2 issue."""
            rz = sb.tile([128, HW], F32, tag="rz", name=f"rz_{i}")
            nc.vector.reciprocal_approx_fast(rz[:], z_ps[:])
            att = sb.tile([128, HW], F32, tag="att", name=f"att_{i}")
            s2 = sb.tile([128, 2], F32, tag="s2", name=f"s2_{i}")
            nc.vector.tensor_tensor(att[:], s_ps[:], rz[:], OP.mult)
            nc.vector.tensor_reduce(s2[:, 0:1], att[:], AX.X, OP.add)
            scr = work.tile([128, HW], BF16, tag="scr", bufs=1, name=f"sq2_{i}")
            nc.scalar.activation(scr[:], att[:], AF.Square, bias=zcol[:],
                                 accum_out=s2[:, 1:2])
            h2 = ar_issue(s2[:], 2, "bn2")
            return att, h2

        def stageD(i, att, h2):
            """AR2 wait, bn2+relu (y-half untransposed), conv3, stats3,
            AR3 issue. Returns (o3b, h3)."""
            g2 = ar_wait(h2, nc.vector)
            a2, b2 = bn_params(nc.vector, g2[:], gb12_sb[:, 2:3],
                               gb12_sb[:, 3:4], "2")
            x2 = sb.tile([128, HW], BF16, tag="x2", name=f"x2_{i}")
            for cc in range(2):
                c0 = cc * 512
                nc.scalar.activation(x2[0:64, c0:c0 + 512],
                                     att[0:64, c0:c0 + 512],
                                     AF.Relu, bias=b2[0:64], scale=a2[0:64])
                # y-half is pixel-transposed; untranspose via the input AP
                nc.scalar.activation(
                    x2[64:128, c0:c0 + 512].rearrange("p (a b) -> p a b", a=16),
                    _sv(att[64:128], cc * 16, [(1, 16), (H, H)]),
                    AF.Relu, bias=b2[64:128], scale=a2[64:128])
            o3b = sb.tile([128, 4, HW], BF16, tag="o3b", name=f"o3b_{i}")
            s3 = sb.tile([128, 8], F32, tag="s3", name=f"s3_{i}")
            for jc in range(4):
                o3_ps = psum.tile([128, HW], F32, tag="TW", name=f"o3_ps_{i}_{jc}")
                for n in range(2):
                    nc.tensor.matmul(
                        o3_ps[:, n * 512:(n + 1) * 512],
                        w3t_sb[:, jc * 128:(jc + 1) * 128],
                        x2[:, n * 512:(n + 1) * 512],
                        start=True, stop=True)
                nc.scalar.activation(o3b[:, jc, :], o3_ps[:], AF.Copy,
                                     accum_out=s3[:, 2 * jc:2 * jc + 1])
                scr3 = work.tile([128, HW], BF16, tag="scr3", bufs=2,
                                 name=f"sq3_{i}_{jc}")
                nc.scalar.activation(scr3[:], o3b[:, jc, :], AF.Square,
                                     bias=zcol[:],
                                     accum_out=s3[:, 2 * jc + 1:2 * jc + 2])
            h3 = ar_issue(s3[:], 8, "bn3")
            return o3b, h3

        def stageE_params(i, h3):
            g3 = ar_wait(h3, nc.vector)
            return bn_params(nc.vector, g3[:], gb3_sb[:, 0:8:2],
                             gb3_sb[:, 1:8:2], "3", n=4)

        def stageE_chunk(i, jc, o3b, xb_sb, a3, b3):
            t3 = work.tile([128, HW], F32, tag="t3", bufs=2, name=f"t3_{i}_{jc}")
            nc.vector.scalar_tensor_tensor(
                t3[:], o3b[:, jc, :], a3[:, jc:jc + 1], xb_sb[:, jc, :],
                OP.mult, OP.add)
            ot = work.tile([128, HW], F32, tag="ot", bufs=2, name=f"ot_{i}_{jc}")
            nc.scalar.activation(ot[:], t3[:], AF.Relu, bias=b3[:, jc:jc + 1])
            nc.sync.dma_start(out_d[jc * 128:(jc + 1) * 128, :], ot[:])

        # ---------------- main pipelined rep loop ----------------
        xb_cur = xb_dma(0)
        o1b, s1 = stageA_conv(0, xb_cur)
        h1 = ar_issue(s1[:], 2, "bn1")
        g1 = ar_wait(h1, nc.vector)
        ab1 = bn_params(nc.vector, g1[:], gb12_sb[:, 0:1], gb12_sb[:, 1:2], "1")
        binfo = stageB2(0, stageB1(0, o1b, ab1))

        pend_D = None            # (i, att, h2, xb) awaiting stageD
        pend_E = None            # (i, o3b, xb_sb, h3) awaiting stageE
        xb_nxt = None
        nbinfo = None
        for i in range(_REPS):
            z_ps = psum.tile([128, HW], F32, tag="TZ", name=f"z_ps_{i}")
            s_ps = psum.tile([128, HW], F32, tag="TS", name=f"s_ps_{i}")
            st = {"eb": {}, "evb": {}}
            e_ab = None
            for j in range(KS + 1):
                if j == 0 and i + 1 < _REPS:
                    xb_nxt = xb_dma(i + 1)
                if j < KS:
                    att_mults(i, j, binfo, st)
                    if j < KS - 1:
                        binfo["build_j"](j + 1)
                if j == 0 and pend_D is not None:
                    pi, patt, ph2, pxb = pend_D
                    po3b, ph3 = stageD(pi, patt, ph2)
                    pend_E = (pi, po3b, pxb, ph3)
                if j >= 1:
                    att_zmm(i, j - 1, st, z_ps)
                    att_ev(i, j - 1, binfo, st)
                    att_smm(i, j - 1, st, s_ps)
                if pend_E is not None:
                    pi, po3b, pxb, ph3 = pend_E
                    if j == 5:
                        e_ab = stageE_params(pi, ph3)
                        stageE_chunk(pi, 0, po3b, pxb, *e_ab)
                    elif j == 6:
                        stageE_chunk(pi, 1, po3b, pxb, *e_ab)
                    elif j == 7:
                        stageE_chunk(pi, 2, po3b, pxb, *e_ab)
                        stageE_chunk(pi, 3, po3b, pxb, *e_ab)
                        pend_E = None
            att, h2 = stageC(i, z_ps, s_ps)
            pend_D = (i, att, h2, xb_cur)
            if i + 1 < _REPS:
                no1ps, ns1 = stageA_conv(i + 1, xb_nxt)
                nh1 = ar_issue(ns1[:], 2, "bn1")
                ng1 = ar_wait(nh1, nc.vector)
                nab1 = bn_params(nc.vector, ng1[:], gb12_sb[:, 0:1],
                                 gb12_sb[:, 1:2], "1")
                binfo = stageB2(i + 1, stageB1(i + 1, no1ps, nab1))
                xb_cur = xb_nxt
        # epilogue: tail of the last rep
        pi, patt, ph2, pxb = pend_D
        po3b, ph3 = stageD(pi, patt, ph2)
        a3, b3 = stageE_params(pi, ph3)
        for jc in range(4):
            stageE_chunk(pi, jc, po3b, pxb, a3, b3)

    nc.compile()
    return nc


_NC = None


def _get_nc():
    global _NC
    if _NC is None:
        _NC = _build_nc()
    return _NC


def _prep_inputs(x, W1, g1, b1, Wq, Wk, bk, Wv, bv, rel_x, rel_y, g2, b2, W3, g3, b3):
    f32 = np.float32
    bf = ml_dtypes.bfloat16

    # channel permutation: new partition -> old channel within the 128 planes
    perm = np.zeros(PLANES, dtype=np.int64)
    for g in range(GROUPS):
        for d in range(D):
            p = g * REL + d if d < REL else 64 + g * REL + (d - REL)
            perm[p] = g * D + d

    W1p = np.ascontiguousarray(W1[perm, :])                     # [128, 512]
    w1t = np.ascontiguousarray(W1p.T).astype(bf)                # [512, 128]

    def block_mat(Wg):
        M = np.zeros((PLANES, PLANES), dtype=f32)
        for po in range(PLANES):
            g = (po % 64) // REL
            o = perm[po] - g * D
            for pi_d in range(D):
                pi = g * REL + pi_d if pi_d < REL else 64 + g * REL + (pi_d - REL)
                M[po, pi] = Wg[g, o, pi_d]
        return M

    mqkv = np.stack([np.ascontiguousarray(block_mat(Wg).T)
                     for Wg in (Wq, Wk, Wv)]).astype(bf)        # [3,128,128] (lhsT)

    bkv = np.stack([bk.reshape(-1)[perm], bv.reshape(-1)[perm]], axis=1).astype(f32)

    # per-j rel columns [128, 7]: x-half f(kh)=f(j), y-half f(kw)=f(j)
    relj = np.zeros((PLANES, KS), dtype=f32)
    for p in range(PLANES):
        g = (p % 64) // REL
        dd = perm[p] - g * D
        for j in range(KS):
            relj[p, j] = rel_x[dd, j, 0] if dd < REL else rel_y[dd - REL, 0, j]

    gb12 = np.stack([g1[perm], b1[perm], g2[perm], b2[perm]], axis=1).astype(f32)

    W3p = np.ascontiguousarray(W3[:, perm])                     # [512, 128]
    w3t = np.ascontiguousarray(W3p.T).astype(bf)                # [128, 512]

    gb3 = np.zeros((PLANES, 8), dtype=f32)
    for j in range(4):
        gb3[:, 2 * j] = g3[j * 128:(j + 1) * 128]
        gb3[:, 2 * j + 1] = b3[j * 128:(j + 1) * 128]

    id128 = np.eye(PLANES, dtype=f32).astype(bf)

    shared = dict(w1t=w1t, mqkv=mqkv, w3t=w3t, bkv=bkv, relj=relj,
                  gb12=gb12, gb3=gb3, id128=id128)
    in_maps = []
    for c in range(N_CORES):
        m = dict(shared)
        m["xb"] = np.ascontiguousarray(x[c].reshape(C_IN, HW)).astype(bf)
        in_maps.append(m)
    return in_maps


def _run(inputs, **kw):
    nc = _get_nc()
    in_maps = _prep_inputs(**inputs)
    res = run_bass_kernel_spmd(nc, in_maps, core_ids=list(range(N_CORES)), **kw)
    out = np.stack([res.results[c]["out"].reshape(C_IN, H, W)
                    for c in range(N_CORES)]).astype(np.float32)
    return out, res


def kernel(**inputs):
    out, _ = _run(inputs)
    return out
